# revision 38
# baseline (speedup 1.0000x reference)
"""LoRA attention kernel for Trainium2, batch-sharded across 8 NeuronCores.

Strategy:
  - Data parallel: batch B=8 -> one batch element per core.
  - LoRA factors are folded into Wqkv on the host (exact algebra, float64):
        q' = q @ (I + Aq Bq)  ==>  Wq' = (I + AB).T @ Wq   (per head)
  - DMA-borne tensors (x, Wqkv, Wv, Wproj) travel as float16; q/k stay fp16
    on-chip (11-bit mantissa ~ fp32r), attention weights and v are bf16
    (exp output range exceeds fp16). All feed the PE at 1 row/cycle.
  - q,k are produced transposed ([head_dim, tokens]) directly from x^T so the
    score matmuls need no on-chip transposes. v is produced in natural layout
    with an extra all-ones column per head, so the attention-value matmul
    accumulates the softmax denominators for free in its last output row.
  - Scores are computed transposed, s[k, q]; softmax normalization is applied
    to the (small) attention output instead of the score matrix.
  - The scalar engine (exp over the full N^2 scores, ~660ns per 512-wide
    tile) slightly oversubscribes the attention phase. Three mitigations:
    head 0's attention interleaves with the v projection (one kt iteration
    per fresh vaug tile), starting the exp stream ~12us early; the
    interleaved next-head q/k projections give per-head catch-up windows;
    and the last two heads (which have no q/k window) run ready
    output-projection partials (c=0..4) on the PE instead of idling,
    shrinking the projection phase by the same amount.
  - Moving operands may be slices of wide tiles (measured: no pitch
    penalty), so weight inputs arrive as few wide DMAs; x arrives as
    12 half-tiles because its first chunks gate the first real matmuls,
    and early transfers fair-share ~295GB/s, completing roughly in issue
    order with ~10us first-byte latency.
  - The PE must never idle mid-kernel: a >~1us gap trips the HAM power
    manager into a >=3.4us half-clock window. Warmup matmuls bridge the
    DMA lead-in, head 0's q/k chain pads between arrival-limited steps,
    and the last head handles its query chunks in reverse so the output
    projection (rotated to start at token tile 4) never waits on it.
"""
import numpy as np

import concourse.bass as bass
import concourse.bacc as bacc
import concourse.mybir as mybir
import concourse.tile as tile
from concourse.bass_utils import run_bass_kernel_spmd

F32 = mybir.dt.float32
F32R = mybir.dt.float32r
F16 = mybir.dt.float16
BF16 = mybir.dt.bfloat16
EXP = mybir.ActivationFunctionType.Exp

B, N, C, H, HD = 8, 1024, 768, 12, 64
CT = C // 128           # 6 contraction tiles over C
TT = N // 128           # 8 token tiles
QC = N // 512           # 2 query chunks of 512
KT = N // 128           # 8 key tiles of 128
EC = 2                  # output-projection feature chunks of 384
SCALE = HD ** -0.5
N_CORES = 8

N_WARMUP = 22           # 256-row PE warmups bridging the DMA lead-in

_NC_CACHE = None


def _build():
    nc = bacc.Bacc(None, target_bir_lowering=False)

    xT = nc.dram_tensor("xT", [C, N], F16, kind="ExternalInput")
    wqk = nc.dram_tensor("wqk", [H, 128, C], F16, kind="ExternalInput")
    wv = nc.dram_tensor("wv", [CT, 128, C], F16, kind="ExternalInput")
    wpt = nc.dram_tensor("wpt", [CT, 128, C], F16, kind="ExternalInput")
    bias = nc.dram_tensor("bias", [1, C], F32, kind="ExternalInput")
    y = nc.dram_tensor("y", [N, C], F32, kind="ExternalOutput")

    from contextlib import ExitStack
    with tile.TileContext(nc) as tc:
        with ExitStack() as ctx:
            pool = lambda name, bufs, **kw: ctx.enter_context(
                tc.tile_pool(name=name, bufs=bufs, **kw))
            xt_pool = pool("xt", 2 * CT)
            wqk_pool = pool("wqkp", 3)
            wv_pool = pool("wvp", 2 * CT)
            wpt_pool = pool("wptp", CT)
            vaug_pool = pool("vaug", TT)
            st_pool = pool("stp", 6)
            kt_pool = pool("ktp", 6)
            exp_pool = pool("expp", 10)
            avs_pool = pool("avsp", 4)
            iv_pool = pool("ivp", 3)
            bc_pool = pool("bcp", 3)
            ost_pool = pool("ostp", 3)
            out_pool = pool("outp", CT)
            y_pool = pool("yp", 3)
            cst_pool = pool("cst", 1)
            proj_ps = pool("proj_ps", 3, space="PSUM")
            sc_ps = pool("sc_ps", 3, space="PSUM")
            av_ps = pool("av_ps", 2, space="PSUM")

            # ---- constants first: the warmup matmuls depend only on wur ----
            wuf = cst_pool.tile([128, 512], F32, tag="wuf")
            nc.vector.memset(wuf, 0.0)
            wur = cst_pool.tile([128, 512], F32R, tag="wur")
            nc.vector.tensor_copy(wur, wuf)
            ones12 = cst_pool.tile([128, H], F32, tag="ones12")
            nc.vector.memset(ones12, 1.0)

            # ---- input loads, issued from three engines in parallel -------
            # (one DMA descriptor costs ~0.6us of engine time; transfers
            # fair-share ~295GB/s, so spread the issues and keep the
            # first-consumed tiles in each engine's first slots)
            nc.sync.dma_start(
                out=(wqkt0 := wqk_pool.tile([128, C], F16, tag="wqk",
                                            name="wqk0")),
                in_=wqk[0, :, :])

            xt = [[None] * QC for _ in range(CT)]
            eng = [nc.sync, nc.scalar, nc.gpsimd]
            for qc in range(QC):
                for c in range(CT):
                    t = xt_pool.tile([128, 512], F16, tag="xt",
                                     name=f"xt{c}_{qc}")
                    eng[c % 3].dma_start(
                        out=t, in_=xT[c * 128:(c + 1) * 128,
                                      qc * 512:(qc + 1) * 512])
                    xt[c][qc] = t

            wvt = [[None] * 2 for _ in range(CT)]
            for half in range(2):
                for c in range(CT):
                    t = wv_pool.tile([128, 384], F16, tag="wv",
                                     name=f"wv{c}_{half}")
                    eng[c % 3].dma_start(
                        out=t, in_=wv[c, :, half * 384:(half + 1) * 384])
                    wvt[c][half] = t

            bias_bc = cst_pool.tile([128, C], F32, tag="biasbc")

            # ---- PE warm-up: dummy matmuls bridge the DMA lead-in so the
            # HAM clock gate opens before real work arrives -----------------
            for i in range(N_WARMUP):
                wps = proj_ps.tile([128, 256], F32, tag="mmps",
                                   name=f"wu_{i}")
                nc.tensor.matmul(wps, wur[:, 0:128], wur[:, 0:256],
                                 start=True, stop=True)

            def load_wqk(h):
                t = wqk_pool.tile([128, C], F16, tag="wqk", name=f"wqk{h}")
                nc.sync.dma_start(out=t, in_=wqk[h, :, :])
                return t

            # ---- per-head q/k projection ---------------------------------
            kt_zeroed = [0]

            def qk_project(h, wqkt, fill=False):
                """q (rows 0-63) and k (rows 64-127), transposed layout.
                Returns ([stA, stB], [ktA, ktB]) per 512-token chunk.
                fill=True (head 0 only): pad between accumulation steps
                with dummy matmuls - the chain runs at the DMA arrival
                frontier (~0.65us/tile vs 0.23us consumption) and a >~1us
                PE gap there costs a 6.8us HAM half-clock window."""
                sts, kts = [], []
                for qc in range(QC):
                    st = st_pool.tile([128, 512], F16, tag="st",
                                      name=f"st{h}_{qc}")
                    pqk = proj_ps.tile([128, 512], F32, tag="mmps",
                                       name=f"pqk{h}_{qc}")
                    for c in range(CT):
                        nc.tensor.matmul(
                            pqk, wqkt[:, c * 128:(c + 1) * 128], xt[c][qc],
                            start=(c == 0), stop=(c == CT - 1),
                        )
                        if fill and c < CT - 1:
                            for r in range(2):
                                wf = sc_ps.tile([128, 256], F32, tag="sc",
                                                name=f"qf{h}_{qc}_{c}_{r}")
                                nc.tensor.matmul(wf, wur[:, 0:128],
                                                 wur[:, 0:256],
                                                 start=True, stop=True)
                    nc.vector.tensor_copy(st, pqk)
                    # move k rows to the top of a base-0 tile (partition shift
                    # via DMA) and zero rows 64-127 so the score matmuls can
                    # run with K=128 (uniform PE tile config; zeros are exact)
                    kt_t = kt_pool.tile([128, 512], F16, tag="kt",
                                        name=f"kt{h}_{qc}")
                    nc.sync.dma_start(out=kt_t[0:64, :], in_=st[64:128, :])
                    # the 6 pool buffers recycle across all heads and only
                    # rows 0:64 are ever rewritten: zero rows 64:128 once
                    # per buffer, the zeros persist in SBUF thereafter
                    if kt_zeroed[0] < 6:
                        nc.vector.tensor_copy(kt_t[64:128, :],
                                              wur[64:128, :])
                        kt_zeroed[0] += 1
                    sts.append(st)
                    kts.append(kt_t)
                return sts, kts

            head_order = list(range(H))
            head_order[10], head_order[11] = head_order[11], head_order[10]

            # ---- v_aug[tt] = [v | 1] per head, natural layout ------------
            vaug = []
            # head 0's q/k runs first (its inputs arrive first); its
            # attention then interleaves with the v projection below - one
            # kt iteration per freshly built vaug tile. This starts the
            # scalar engine's exp stream ~12us earlier (it is the global
            # attention constraint) and fills the v-projection's
            # DMA-arrival gaps with compute that needs no new data.
            sts0, kts0 = qk_project(0, wqkt0, fill=True)
            wqk_next = load_wqk(head_order[1])
            av0 = [av_ps.tile([128, 512], F32, tag="av", name=f"av0_{qc}")
                   for qc in range(QC)]

            def attn0_iter(kt):
                for qc in range(QC):
                    ps_s = sc_ps.tile([128, 512], F32, tag="sc",
                                      name=f"sc0_{qc}_{kt}")
                    nc.tensor.matmul(
                        ps_s,
                        kts0[kt // 4][:, (kt % 4) * 128:(kt % 4 + 1) * 128],
                        sts0[qc], start=True, stop=True,
                    )
                    et = exp_pool.tile([128, 512], BF16, tag="exp",
                                       name=f"exp0_{qc}_{kt}")
                    nc.scalar.activation(out=et, in_=ps_s, func=EXP,
                                         scale=SCALE)
                    nc.tensor.matmul(
                        av0[qc], vaug[kt][:, 0:128], et,
                        start=(kt == 0), stop=(kt == KT - 1),
                    )

            for tt in range(TT):
                va = vaug_pool.tile([128, (H - 1) * 65 + 128], BF16,
                                    tag="vaug", name=f"vaug{tt}")
                for half in range(2):
                    pv = proj_ps.tile([128, 384], F32, tag="mmps",
                                      name=f"pv{tt}_{half}")
                    for c in range(CT):
                        nc.tensor.matmul(
                            pv,
                            xt[c][tt // 4][:, (tt % 4) * 128:(tt % 4 + 1) * 128],
                            wvt[c][half],
                            start=(c == 0), stop=(c == CT - 1),
                        )
                    dst = bass.AP(tensor=va.tensor,
                                  offset=va.offset + half * 6 * 65,
                                  ap=[va.ap[0], [65, 6], [1, 64]])
                    nc.vector.tensor_copy(dst, pv)
                ones_ap = bass.AP(tensor=va.tensor, offset=va.offset + 64,
                                  ap=[va.ap[0], [65, H]])
                nc.vector.tensor_copy(ones_ap, ones12)
                # zero the tail cols so the widened av lhsT reads no garbage
                nc.vector.tensor_copy(va[:, H * 65:], wur[:, 0:(H - 1) * 65 + 128 - H * 65])
                vaug.append(va)
                # head 0's attention lags one token tile behind the v
                # construction: av(kt) then never waits on the current
                # tile's vector copies (~1.9us of DVE work per tile)
                if tt >= 1:
                    attn0_iter(tt - 1)
            # the next head's q/k chains go ahead of the trailing
            # iteration: they are independent, and their ~2.8us of PE work
            # covers the bare score->exp->av latency of kt=7
            qk1_pre = qk_project(head_order[1], wqk_next)
            wqk_next = load_wqk(head_order[2])
            attn0_iter(KT - 1)

            # ---- output accumulator tiles (c-major, [128, N]) ------------
            outT = [out_pool.tile([128, N], F16, tag="outT", name=f"outT{i}")
                    for i in range(CT)]

            def attn_norm(h, qc, av):
                # drain the psum to release the bank; row 64 = softmax
                # denominators. DMA-shift them to partition 0, then
                # fast-reciprocal and broadcast (both need base partition 0).
                avs = avs_pool.tile([65, 512], F32, tag="avs",
                                    name=f"avs{h}_{qc}")
                nc.vector.tensor_copy(avs, av[0:65, :])
                sm0 = iv_pool.tile([1, 512], F32, tag="sm0",
                                   name=f"sm0{h}_{qc}")
                nc.sync.dma_start(out=sm0, in_=avs[64:65, :])
                iv0 = iv_pool.tile([1, 512], F32, tag="iv0",
                                   name=f"iv0{h}_{qc}")
                nc.vector.reciprocal_approx_fast(out=iv0, in_=sm0)
                bc = bc_pool.tile([64, 512], F32, tag="bc",
                                  name=f"bc{h}_{qc}")
                nc.gpsimd.partition_broadcast(bc, iv0)
                ct_i = h // 2
                if h % 2 == 0:
                    nc.vector.tensor_mul(
                        outT[ct_i][0:64, qc * 512:(qc + 1) * 512],
                        avs[0:64, :], bc)
                else:
                    ost = ost_pool.tile([64, 512], F16, tag="ost",
                                        name=f"ost{h}_{qc}")
                    nc.vector.tensor_mul(ost, avs[0:64, :], bc)
                    nc.sync.dma_start(
                        out=outT[ct_i][64:128, qc * 512:(qc + 1) * 512],
                        in_=ost)

            for qc in range(QC):
                attn_norm(0, qc, av0[qc])

            # ---- prefilled output-projection chains ----------------------
            # (tt,ec) groups whose c=0..4 partials run inside the last two
            # heads' attention windows; c=5 closes them in the proj phase
            prefill = {(tt, ec): None for tt in (4, 5, 6) for ec in (0, 1)}
            wptt = [None] * CT

            def py_open(tt, ec):
                py = proj_ps.tile([128, 384], F32, tag="mmps",
                                  name=f"py{tt}_{ec}")
                prefill[(tt, ec)] = py
                return py

            def py_step(py, tt, ec, c, start, stop):
                nc.tensor.matmul(
                    py, outT[c][:, tt * 128:(tt + 1) * 128],
                    wptt[c][:, ec * 384:(ec + 1) * 384],
                    start=start, stop=stop,
                )

            fillers = []                 # deferred (tt, ec, c) filler steps
            # only 3 groups may be open at once (proj_ps bufs=3); their c=5
            # closes come later in the PE stream, so a 4th would deadlock.
            # Ordered so only groups (4,0),(4,1) open during the
            # second-to-last head (the last head's q/k projection still
            # needs proj_ps buffers), and the c=4 steps - whose outT[4]
            # qc1 input lands only at the end of that head - come last.
            for c in range(4):
                for tt, ec in ((4, 0), (4, 1)):
                    fillers.append((tt, ec, c))
            for c in range(4):
                fillers.append((5, 0, c))
            for tt, ec in ((4, 0), (4, 1), (5, 0)):
                fillers.append((tt, ec, 4))
            fill_i = 0

            def emit_fillers(n):
                nonlocal fill_i
                for _ in range(n):
                    if fill_i >= len(fillers):
                        return
                    tt, ec, c = fillers[fill_i]
                    py = prefill[(tt, ec)] if c else py_open(tt, ec)
                    py_step(py, tt, ec, c, start=(c == 0), stop=False)
                    fill_i += 1

            # ---- per-head attention (head 0 already done above) ----------
            for pos, h in enumerate(head_order):
                if pos == 0:
                    continue
                sts, kts = qk1_pre if pos == 1 else \
                    qk_project(h, wqk_next)
                if 2 <= pos < H - 1:
                    # prefetch the next head's weights (issued here, one
                    # full head-period before use; kept out of the
                    # DMA-critical lead-in window)
                    wqk_next = load_wqk(head_order[pos + 1])

                # the LAST head's qc1 outputs gate the (rotated) projection:
                # do qc1 first there so its normalization chain overlaps the
                # qc0 attention matmuls
                qc_order = (1, 0) if pos == H - 1 else (0, 1)
                for qc in qc_order:
                    av = av_ps.tile([128, 512], F32, tag="av",
                                    name=f"av{h}_{qc}")
                    for kt in range(KT):
                        ps_s = sc_ps.tile([128, 512], F32, tag="sc",
                                          name=f"sc{h}_{qc}_{kt}")
                        nc.tensor.matmul(
                            ps_s,
                            kts[kt // 4][:, (kt % 4) * 128:(kt % 4 + 1) * 128],
                            sts[qc], start=True, stop=True,
                        )
                        et = exp_pool.tile([128, 512], BF16, tag="exp",
                                           name=f"exp{h}_{qc}_{kt}")
                        nc.scalar.activation(out=et, in_=ps_s, func=EXP,
                                             scale=SCALE)
                        nc.tensor.matmul(
                            av, vaug[kt][:, h * 65:h * 65 + 128], et,
                            start=(kt == 0), stop=(kt == KT - 1),
                        )
                        # the last two heads have no interleaved q/k work;
                        # give the scalar engine catch-up room by running
                        # ready projection partials on the PE instead
                        if pos >= H - 2 and kt % 2 == 1:
                            emit_fillers(1)
                    attn_norm(h, qc, av)

                if 2 <= pos <= 7:
                    # prefetch output-projection weights mid-flight, one
                    # wide tile per head so no queue backs up
                    c = pos - 2
                    t = wpt_pool.tile([128, C], F16, tag="wpt",
                                      name=f"wpt{c}")
                    nc.sync.dma_start(out=t, in_=wpt[c, :, :])
                    wptt[c] = t
                    if pos == 2:
                        # bias is only needed by the projection; keep its
                        # 384KB out of the DMA-critical lead-in
                        nc.gpsimd.dma_start(
                            out=bias_bc,
                            in_=bias[:, :].to_broadcast([128, C]))

            # zero-dep matmuls bridge the attention->projection seam: the
            # first py closes wait on the last head's qc1 normalization,
            # which lands ~1.7us after its final attention matmul
            for i in range(6):
                wg = sc_ps.tile([128, 512], F32, tag="sc", name=f"sg_{i}")
                nc.tensor.matmul(wg, wur[:, 0:128], wur,
                                 start=True, stop=True)

            # ---- output projection ---------------------------------------
            # rotated: tt 4-7 first (they need the last head's qc1 outputs,
            # done first there); tt 0-3 need its qc0 outputs, which land
            # while tt 4-7 are on the PE. Prefilled groups just close (c=5).
            ysbs = {}
            for tt in [4, 5, 6, 7, 0, 1, 2, 3]:
                ysb = y_pool.tile([128, C], F32, tag="y", name=f"y{tt}")
                for ec in range(EC):
                    py = prefill.get((tt, ec))
                    if py is not None:
                        py_step(py, tt, ec, 5, start=False, stop=True)
                    else:
                        py = py_open(tt, ec) if (tt, ec) in prefill else \
                            proj_ps.tile([128, 384], F32, tag="mmps",
                                         name=f"py{tt}_{ec}")
                        for c in range(CT):
                            py_step(py, tt, ec, c, start=(c == 0),
                                    stop=(c == CT - 1))
                    nc.vector.tensor_add(ysb[:, ec * 384:(ec + 1) * 384], py,
                                         bias_bc[:, ec * 384:(ec + 1) * 384])
                    if tt == 3:
                        # split the final tile's writeback so the last DMA
                        # only covers half the tile
                        nc.sync.dma_start(
                            out=y[tt * 128:(tt + 1) * 128,
                                  ec * 384:(ec + 1) * 384],
                            in_=ysb[:, ec * 384:(ec + 1) * 384])
                if tt != 3:
                    nc.sync.dma_start(out=y[tt * 128:(tt + 1) * 128, :],
                                      in_=ysb)

    nc.finalize()
    return nc


def _get_nc():
    global _NC_CACHE
    if _NC_CACHE is None:
        _NC_CACHE = _build()
    return _NC_CACHE


def _host_prep(x, Wqkv, Wproj, bproj, Aq, Bq, Av, Bv):
    """Fold LoRA into the weights and lay everything out for the kernel."""
    W = Wqkv.astype(np.float64)
    Wq = W[0:C].reshape(H, HD, C)
    Wk = W[C:2 * C].reshape(H, HD, C)
    Wv_ = W[2 * C:3 * C].reshape(H, HD, C)
    ABq = Aq.astype(np.float64) @ Bq.astype(np.float64)   # [HD, HD]
    ABv = Av.astype(np.float64) @ Bv.astype(np.float64)
    Wq = Wq + np.einsum('ed,hec->hdc', ABq, Wq)           # (I+AB).T @ Wq per head
    Wv_ = Wv_ + np.einsum('ed,hec->hdc', ABv, Wv_)

    # wqk[h] = [K = c-rows(128), 6 col-blocks of (q_h cols(64) ++ k_h cols(64))]
    wqk = np.empty((H, 128, C), np.float16)
    for h in range(H):
        for c in range(CT):
            cs = slice(c * 128, (c + 1) * 128)
            wqk[h, :, c * 128:c * 128 + 64] = Wq[h][:, cs].T.astype(np.float16)
            wqk[h, :, c * 128 + 64:c * 128 + 128] = \
                Wk[h][:, cs].T.astype(np.float16)

    # wv[c] = [K=c-rows(128), all 768 v output features]
    WvT = Wv_.reshape(C, C).T.astype(np.float16)          # [c_in, v_out]
    wv = np.ascontiguousarray(WvT.reshape(CT, 128, C))

    # wpt[c] = Wproj.T c-tiles: [K=c(128), e(768)]
    WpT = Wproj.astype(np.float16).T                      # [c, e]
    wpt = np.ascontiguousarray(WpT.reshape(CT, 128, C))

    bias = bproj.astype(np.float32).reshape(1, C)

    per_core = []
    for b in range(B):
        xTb = np.ascontiguousarray(x[b].astype(np.float16).T)   # [C, N]
        per_core.append({"xT": xTb, "wqk": wqk, "wv": wv, "wpt": wpt,
                         "bias": bias})
    return per_core


def kernel(x, Wqkv, Wproj, bproj, Aq, Bq, Av, Bv, _trace=False):
    x = np.asarray(x)
    in_maps = _host_prep(np.asarray(x), np.asarray(Wqkv), np.asarray(Wproj),
                         np.asarray(bproj), np.asarray(Aq), np.asarray(Bq),
                         np.asarray(Av), np.asarray(Bv))
    nc = _get_nc()
    res = run_bass_kernel_spmd(nc, in_maps, core_ids=list(range(N_CORES)),
                               trace=_trace)
    out = np.stack([np.asarray(res.results[b]["y"]) for b in range(B)], axis=0)
    if _trace:
        kernel._last_result = res
    return out.astype(np.float32)


# revision 40
# speedup vs baseline: 1.0044x; 1.0044x over previous
"""LoRA attention kernel for Trainium2, batch-sharded across 8 NeuronCores.

Strategy:
  - Data parallel: batch B=8 -> one batch element per core.
  - LoRA factors are folded into Wqkv on the host (exact algebra, float64):
        q' = q @ (I + Aq Bq)  ==>  Wq' = (I + AB).T @ Wq   (per head)
  - DMA-borne tensors (x, Wqkv, Wv, Wproj) travel as float16; q/k stay fp16
    on-chip (11-bit mantissa ~ fp32r), attention weights and v are bf16
    (exp output range exceeds fp16). All feed the PE at 1 row/cycle.
  - q,k are produced transposed ([head_dim, tokens]) directly from x^T so the
    score matmuls need no on-chip transposes. v is produced in natural layout
    with an extra all-ones column per head, so the attention-value matmul
    accumulates the softmax denominators for free in its last output row.
  - Scores are computed transposed, s[k, q]; softmax normalization is applied
    to the (small) attention output instead of the score matrix.
  - The scalar engine (exp over the full N^2 scores, ~660ns per 512-wide
    tile) slightly oversubscribes the attention phase. Three mitigations:
    head 0's attention interleaves with the v projection (one kt iteration
    per fresh vaug tile), starting the exp stream ~12us early; the
    interleaved next-head q/k projections give per-head catch-up windows;
    and the last two heads (which have no q/k window) run ready
    output-projection partials (c=0..4) on the PE instead of idling,
    shrinking the projection phase by the same amount.
  - Moving operands may be slices of wide tiles (measured: no pitch
    penalty), so weight inputs arrive as few wide DMAs; x arrives as
    12 half-tiles because its first chunks gate the first real matmuls,
    and early transfers fair-share ~295GB/s, completing roughly in issue
    order with ~10us first-byte latency.
  - The PE must never idle mid-kernel: a >~1us gap trips the HAM power
    manager into a >=3.4us half-clock window. Warmup matmuls bridge the
    DMA lead-in, head 0's q/k chain pads between arrival-limited steps,
    and the last head handles its query chunks in reverse so the output
    projection (rotated to start at token tile 4) never waits on it.
"""
import numpy as np

import concourse.bass as bass
import concourse.bacc as bacc
import concourse.mybir as mybir
import concourse.tile as tile
from concourse.bass_utils import run_bass_kernel_spmd

F32 = mybir.dt.float32
F32R = mybir.dt.float32r
F16 = mybir.dt.float16
BF16 = mybir.dt.bfloat16
EXP = mybir.ActivationFunctionType.Exp

B, N, C, H, HD = 8, 1024, 768, 12, 64
CT = C // 128           # 6 contraction tiles over C
TT = N // 128           # 8 token tiles
QC = N // 512           # 2 query chunks of 512
KT = N // 128           # 8 key tiles of 128
EC = 2                  # output-projection feature chunks of 384
SCALE = HD ** -0.5
N_CORES = 8

N_WARMUP = 22           # 256-row PE warmups bridging the DMA lead-in

_NC_CACHE = None


def _build():
    nc = bacc.Bacc(None, target_bir_lowering=False)

    xT = nc.dram_tensor("xT", [C, N], F16, kind="ExternalInput")
    wqk = nc.dram_tensor("wqk", [H, 128, C], F16, kind="ExternalInput")
    wv = nc.dram_tensor("wv", [CT, 128, C], F16, kind="ExternalInput")
    wpt = nc.dram_tensor("wpt", [CT, 128, C], F16, kind="ExternalInput")
    bias = nc.dram_tensor("bias", [1, C], F32, kind="ExternalInput")
    y = nc.dram_tensor("y", [N, C], F32, kind="ExternalOutput")

    from contextlib import ExitStack
    with tile.TileContext(nc) as tc:
        with ExitStack() as ctx:
            pool = lambda name, bufs, **kw: ctx.enter_context(
                tc.tile_pool(name=name, bufs=bufs, **kw))
            xt_pool = pool("xt", 2 * CT)
            wqk_pool = pool("wqkp", 3)
            wv_pool = pool("wvp", 2 * CT)
            wpt_pool = pool("wptp", CT)
            vaug_pool = pool("vaug", TT)
            st_pool = pool("stp", 6)
            kt_pool = pool("ktp", 6)
            exp_pool = pool("expp", 10)
            avs_pool = pool("avsp", 4)
            iv_pool = pool("ivp", 3)
            bc_pool = pool("bcp", 3)
            ost_pool = pool("ostp", 3)
            out_pool = pool("outp", CT)
            y_pool = pool("yp", 3)
            cst_pool = pool("cst", 1)
            proj_ps = pool("proj_ps", 3, space="PSUM")
            sc_ps = pool("sc_ps", 3, space="PSUM")
            av_ps = pool("av_ps", 2, space="PSUM")

            # ---- constants first: the warmup matmuls depend only on wur ----
            wuf = cst_pool.tile([128, 512], F32, tag="wuf")
            nc.vector.memset(wuf, 0.0)
            wur = cst_pool.tile([128, 512], F32R, tag="wur")
            nc.vector.tensor_copy(wur, wuf)
            ones12 = cst_pool.tile([128, H], F32, tag="ones12")
            nc.vector.memset(ones12, 1.0)

            # ---- input loads, issued from three engines in parallel -------
            # (one DMA descriptor costs ~0.6us of engine time; transfers
            # fair-share ~295GB/s, so spread the issues and keep the
            # first-consumed tiles in each engine's first slots)
            nc.sync.dma_start(
                out=(wqkt0 := wqk_pool.tile([128, C], F16, tag="wqk",
                                            name="wqk0")),
                in_=wqk[0, :, :])

            xt = [[None] * QC for _ in range(CT)]
            eng = [nc.sync, nc.scalar, nc.gpsimd]
            for qc in range(QC):
                for c in range(CT):
                    t = xt_pool.tile([128, 512], F16, tag="xt",
                                     name=f"xt{c}_{qc}")
                    eng[c % 3].dma_start(
                        out=t, in_=xT[c * 128:(c + 1) * 128,
                                      qc * 512:(qc + 1) * 512])
                    xt[c][qc] = t

            wvt = [[None] * 2 for _ in range(CT)]
            for half in range(2):
                for c in range(CT):
                    t = wv_pool.tile([128, 384], F16, tag="wv",
                                     name=f"wv{c}_{half}")
                    eng[c % 3].dma_start(
                        out=t, in_=wv[c, :, half * 384:(half + 1) * 384])
                    wvt[c][half] = t

            bias_bc = cst_pool.tile([128, C], F32, tag="biasbc")

            # ---- PE warm-up: dummy matmuls bridge the DMA lead-in so the
            # HAM clock gate opens before real work arrives -----------------
            for i in range(N_WARMUP):
                wps = proj_ps.tile([128, 256], F32, tag="mmps",
                                   name=f"wu_{i}")
                nc.tensor.matmul(wps, wur[:, 0:128], wur[:, 0:256],
                                 start=True, stop=True)

            def load_wqk(h):
                t = wqk_pool.tile([128, C], F16, tag="wqk", name=f"wqk{h}")
                nc.sync.dma_start(out=t, in_=wqk[h, :, :])
                return t

            # ---- per-head q/k projection ---------------------------------
            def qk_project(h, wqkt, fill=False):
                """q (rows 0-63) and k (rows 64-127), transposed layout.
                Returns ([stA, stB], [ktA, ktB]) per 512-token chunk.
                fill=True (head 0 only): pad between accumulation steps
                with dummy matmuls - the chain runs at the DMA arrival
                frontier (~0.65us/tile vs 0.23us consumption) and a >~1us
                PE gap there costs a 6.8us HAM half-clock window."""
                sts, kts = [], []
                for qc in range(QC):
                    st = st_pool.tile([128, 512], F16, tag="st",
                                      name=f"st{h}_{qc}")
                    pqk = proj_ps.tile([128, 512], F32, tag="mmps",
                                       name=f"pqk{h}_{qc}")
                    for c in range(CT):
                        nc.tensor.matmul(
                            pqk, wqkt[:, c * 128:(c + 1) * 128], xt[c][qc],
                            start=(c == 0), stop=(c == CT - 1),
                        )
                        if fill and c < CT - 1:
                            for r in range(2):
                                wf = sc_ps.tile([128, 256], F32, tag="sc",
                                                name=f"qf{h}_{qc}_{c}_{r}")
                                nc.tensor.matmul(wf, wur[:, 0:128],
                                                 wur[:, 0:256],
                                                 start=True, stop=True)
                    nc.vector.tensor_copy(st, pqk)
                    # move k rows to the top of a base-0 tile (partition shift
                    # via DMA) and zero rows 64-127 so the score matmuls can
                    # run with K=128 (uniform PE tile config; zeros are exact)
                    kt_t = kt_pool.tile([128, 512], F16, tag="kt",
                                        name=f"kt{h}_{qc}")
                    nc.sync.dma_start(out=kt_t[0:64, :], in_=st[64:128, :])
                    nc.vector.tensor_copy(kt_t[64:128, :], wur[64:128, :])
                    sts.append(st)
                    kts.append(kt_t)
                return sts, kts

            head_order = list(range(H))
            head_order[10], head_order[11] = head_order[11], head_order[10]

            # ---- v_aug[tt] = [v | 1] per head, natural layout ------------
            vaug = []
            # head 0's q/k runs first (its inputs arrive first); its
            # attention then interleaves with the v projection below - one
            # kt iteration per freshly built vaug tile. This starts the
            # scalar engine's exp stream ~12us earlier (it is the global
            # attention constraint) and fills the v-projection's
            # DMA-arrival gaps with compute that needs no new data.
            sts0, kts0 = qk_project(0, wqkt0, fill=True)
            wqk_next = load_wqk(head_order[1])
            av0 = [av_ps.tile([128, 512], F32, tag="av", name=f"av0_{qc}")
                   for qc in range(QC)]

            def attn0_iter(kt):
                for qc in range(QC):
                    ps_s = sc_ps.tile([128, 512], F32, tag="sc",
                                      name=f"sc0_{qc}_{kt}")
                    nc.tensor.matmul(
                        ps_s,
                        kts0[kt // 4][:, (kt % 4) * 128:(kt % 4 + 1) * 128],
                        sts0[qc], start=True, stop=True,
                    )
                    et = exp_pool.tile([128, 512], BF16, tag="exp",
                                       name=f"exp0_{qc}_{kt}")
                    nc.scalar.activation(out=et, in_=ps_s, func=EXP,
                                         scale=SCALE)
                    nc.tensor.matmul(
                        av0[qc], vaug[kt][:, 0:128], et,
                        start=(kt == 0), stop=(kt == KT - 1),
                    )

            for tt in range(TT):
                va = vaug_pool.tile([128, (H - 1) * 65 + 128], BF16,
                                    tag="vaug", name=f"vaug{tt}")
                for half in range(2):
                    pv = proj_ps.tile([128, 384], F32, tag="mmps",
                                      name=f"pv{tt}_{half}")
                    for c in range(CT):
                        nc.tensor.matmul(
                            pv,
                            xt[c][tt // 4][:, (tt % 4) * 128:(tt % 4 + 1) * 128],
                            wvt[c][half],
                            start=(c == 0), stop=(c == CT - 1),
                        )
                    dst = bass.AP(tensor=va.tensor,
                                  offset=va.offset + half * 6 * 65,
                                  ap=[va.ap[0], [65, 6], [1, 64]])
                    nc.vector.tensor_copy(dst, pv)
                ones_ap = bass.AP(tensor=va.tensor, offset=va.offset + 64,
                                  ap=[va.ap[0], [65, H]])
                nc.vector.tensor_copy(ones_ap, ones12)
                # zero the tail cols so the widened av lhsT reads no garbage
                nc.vector.tensor_copy(va[:, H * 65:], wur[:, 0:(H - 1) * 65 + 128 - H * 65])
                vaug.append(va)
                # head 0's attention lags one token tile behind the v
                # construction: av(kt) then never waits on the current
                # tile's vector copies (~1.9us of DVE work per tile)
                if tt >= 1:
                    attn0_iter(tt - 1)
            # the next head's q/k chains go ahead of the trailing
            # iteration: they are independent, and their ~2.8us of PE work
            # covers the bare score->exp->av latency of kt=7
            qk1_pre = qk_project(head_order[1], wqk_next)
            wqk_next = load_wqk(head_order[2])
            attn0_iter(KT - 1)

            # ---- output accumulator tiles (c-major, [128, N]) ------------
            outT = [out_pool.tile([128, N], F16, tag="outT", name=f"outT{i}")
                    for i in range(CT)]

            def attn_norm(h, qc, av):
                # drain the psum to release the bank; row 64 = softmax
                # denominators. DMA-shift them to partition 0, then
                # fast-reciprocal and broadcast (both need base partition 0).
                avs = avs_pool.tile([65, 512], F32, tag="avs",
                                    name=f"avs{h}_{qc}")
                nc.vector.tensor_copy(avs, av[0:65, :])
                sm0 = iv_pool.tile([1, 512], F32, tag="sm0",
                                   name=f"sm0{h}_{qc}")
                # issue from gpsimd: sync's queue carries 192KB weight
                # prefetches mid-head that would delay this tiny shift,
                # and gpsimd owns the dependent broadcast anyway
                nc.gpsimd.dma_start(out=sm0, in_=avs[64:65, :])
                iv0 = iv_pool.tile([1, 512], F32, tag="iv0",
                                   name=f"iv0{h}_{qc}")
                nc.vector.reciprocal_approx_fast(out=iv0, in_=sm0)
                bc = bc_pool.tile([64, 512], F32, tag="bc",
                                  name=f"bc{h}_{qc}")
                nc.gpsimd.partition_broadcast(bc, iv0)
                ct_i = h // 2
                if h % 2 == 0:
                    nc.vector.tensor_mul(
                        outT[ct_i][0:64, qc * 512:(qc + 1) * 512],
                        avs[0:64, :], bc)
                else:
                    ost = ost_pool.tile([64, 512], F16, tag="ost",
                                        name=f"ost{h}_{qc}")
                    nc.vector.tensor_mul(ost, avs[0:64, :], bc)
                    nc.sync.dma_start(
                        out=outT[ct_i][64:128, qc * 512:(qc + 1) * 512],
                        in_=ost)

            for qc in range(QC):
                attn_norm(0, qc, av0[qc])

            # ---- prefilled output-projection chains ----------------------
            # (tt,ec) groups whose c=0..4 partials run inside the last two
            # heads' attention windows; c=5 closes them in the proj phase
            prefill = {(tt, ec): None for tt in (4, 5, 6) for ec in (0, 1)}
            wptt = [None] * CT

            def py_open(tt, ec):
                py = proj_ps.tile([128, 384], F32, tag="mmps",
                                  name=f"py{tt}_{ec}")
                prefill[(tt, ec)] = py
                return py

            def py_step(py, tt, ec, c, start, stop):
                nc.tensor.matmul(
                    py, outT[c][:, tt * 128:(tt + 1) * 128],
                    wptt[c][:, ec * 384:(ec + 1) * 384],
                    start=start, stop=stop,
                )

            fillers = []                 # deferred (tt, ec, c) filler steps
            # only 3 groups may be open at once (proj_ps bufs=3); their c=5
            # closes come later in the PE stream, so a 4th would deadlock.
            # Ordered so only groups (4,0),(4,1) open during the
            # second-to-last head (the last head's q/k projection still
            # needs proj_ps buffers), and the c=4 steps - whose outT[4]
            # qc1 input lands only at the end of that head - come last.
            for c in range(4):
                for tt, ec in ((4, 0), (4, 1)):
                    fillers.append((tt, ec, c))
            for c in range(4):
                fillers.append((5, 0, c))
            for tt, ec in ((4, 0), (4, 1), (5, 0)):
                fillers.append((tt, ec, 4))
            fill_i = 0

            def emit_fillers(n):
                nonlocal fill_i
                for _ in range(n):
                    if fill_i >= len(fillers):
                        return
                    tt, ec, c = fillers[fill_i]
                    py = prefill[(tt, ec)] if c else py_open(tt, ec)
                    py_step(py, tt, ec, c, start=(c == 0), stop=False)
                    fill_i += 1

            # ---- per-head attention (head 0 already done above) ----------
            for pos, h in enumerate(head_order):
                if pos == 0:
                    continue
                sts, kts = qk1_pre if pos == 1 else \
                    qk_project(h, wqk_next)
                if 2 <= pos < H - 1:
                    # prefetch the next head's weights (issued here, one
                    # full head-period before use; kept out of the
                    # DMA-critical lead-in window)
                    wqk_next = load_wqk(head_order[pos + 1])

                # the LAST head's qc1 outputs gate the (rotated) projection:
                # do qc1 first there so its normalization chain overlaps the
                # qc0 attention matmuls
                qc_order = (1, 0) if pos == H - 1 else (0, 1)
                for qc in qc_order:
                    av = av_ps.tile([128, 512], F32, tag="av",
                                    name=f"av{h}_{qc}")
                    for kt in range(KT):
                        ps_s = sc_ps.tile([128, 512], F32, tag="sc",
                                          name=f"sc{h}_{qc}_{kt}")
                        nc.tensor.matmul(
                            ps_s,
                            kts[kt // 4][:, (kt % 4) * 128:(kt % 4 + 1) * 128],
                            sts[qc], start=True, stop=True,
                        )
                        et = exp_pool.tile([128, 512], BF16, tag="exp",
                                           name=f"exp{h}_{qc}_{kt}")
                        nc.scalar.activation(out=et, in_=ps_s, func=EXP,
                                             scale=SCALE)
                        nc.tensor.matmul(
                            av, vaug[kt][:, h * 65:h * 65 + 128], et,
                            start=(kt == 0), stop=(kt == KT - 1),
                        )
                        # the last two heads have no interleaved q/k work;
                        # give the scalar engine catch-up room by running
                        # ready projection partials on the PE instead
                        if pos >= H - 2 and kt % 2 == 1:
                            emit_fillers(1)
                    attn_norm(h, qc, av)

                if 2 <= pos <= 7:
                    # prefetch output-projection weights mid-flight, one
                    # wide tile per head so no queue backs up
                    c = pos - 2
                    t = wpt_pool.tile([128, C], F16, tag="wpt",
                                      name=f"wpt{c}")
                    nc.sync.dma_start(out=t, in_=wpt[c, :, :])
                    wptt[c] = t
                    if pos == 2:
                        # bias is only needed by the projection; keep its
                        # 384KB out of the DMA-critical lead-in
                        nc.gpsimd.dma_start(
                            out=bias_bc,
                            in_=bias[:, :].to_broadcast([128, C]))

            # zero-dep matmuls bridge the attention->projection seam: the
            # first py closes wait on the last head's qc1 normalization,
            # which lands ~1.7us after its final attention matmul
            for i in range(6):
                wg = sc_ps.tile([128, 512], F32, tag="sc", name=f"sg_{i}")
                nc.tensor.matmul(wg, wur[:, 0:128], wur,
                                 start=True, stop=True)

            # ---- output projection ---------------------------------------
            # rotated: tt 4-7 first (they need the last head's qc1 outputs,
            # done first there); tt 0-3 need its qc0 outputs, which land
            # while tt 4-7 are on the PE. Prefilled groups just close (c=5).
            ysbs = {}
            for tt in [4, 5, 6, 7, 0, 1, 2, 3]:
                ysb = y_pool.tile([128, C], F32, tag="y", name=f"y{tt}")
                for ec in range(EC):
                    py = prefill.get((tt, ec))
                    if py is not None:
                        py_step(py, tt, ec, 5, start=False, stop=True)
                    else:
                        py = py_open(tt, ec) if (tt, ec) in prefill else \
                            proj_ps.tile([128, 384], F32, tag="mmps",
                                         name=f"py{tt}_{ec}")
                        for c in range(CT):
                            py_step(py, tt, ec, c, start=(c == 0),
                                    stop=(c == CT - 1))
                    nc.vector.tensor_add(ysb[:, ec * 384:(ec + 1) * 384], py,
                                         bias_bc[:, ec * 384:(ec + 1) * 384])
                    if tt == 3:
                        # split the final tile's writeback so the last DMA
                        # only covers half the tile
                        nc.sync.dma_start(
                            out=y[tt * 128:(tt + 1) * 128,
                                  ec * 384:(ec + 1) * 384],
                            in_=ysb[:, ec * 384:(ec + 1) * 384])
                if tt != 3:
                    nc.sync.dma_start(out=y[tt * 128:(tt + 1) * 128, :],
                                      in_=ysb)

    nc.finalize()
    return nc


def _get_nc():
    global _NC_CACHE
    if _NC_CACHE is None:
        _NC_CACHE = _build()
    return _NC_CACHE


def _host_prep(x, Wqkv, Wproj, bproj, Aq, Bq, Av, Bv):
    """Fold LoRA into the weights and lay everything out for the kernel."""
    W = Wqkv.astype(np.float64)
    Wq = W[0:C].reshape(H, HD, C)
    Wk = W[C:2 * C].reshape(H, HD, C)
    Wv_ = W[2 * C:3 * C].reshape(H, HD, C)
    ABq = Aq.astype(np.float64) @ Bq.astype(np.float64)   # [HD, HD]
    ABv = Av.astype(np.float64) @ Bv.astype(np.float64)
    Wq = Wq + np.einsum('ed,hec->hdc', ABq, Wq)           # (I+AB).T @ Wq per head
    Wv_ = Wv_ + np.einsum('ed,hec->hdc', ABv, Wv_)

    # wqk[h] = [K = c-rows(128), 6 col-blocks of (q_h cols(64) ++ k_h cols(64))]
    wqk = np.empty((H, 128, C), np.float16)
    for h in range(H):
        for c in range(CT):
            cs = slice(c * 128, (c + 1) * 128)
            wqk[h, :, c * 128:c * 128 + 64] = Wq[h][:, cs].T.astype(np.float16)
            wqk[h, :, c * 128 + 64:c * 128 + 128] = \
                Wk[h][:, cs].T.astype(np.float16)

    # wv[c] = [K=c-rows(128), all 768 v output features]
    WvT = Wv_.reshape(C, C).T.astype(np.float16)          # [c_in, v_out]
    wv = np.ascontiguousarray(WvT.reshape(CT, 128, C))

    # wpt[c] = Wproj.T c-tiles: [K=c(128), e(768)]
    WpT = Wproj.astype(np.float16).T                      # [c, e]
    wpt = np.ascontiguousarray(WpT.reshape(CT, 128, C))

    bias = bproj.astype(np.float32).reshape(1, C)

    per_core = []
    for b in range(B):
        xTb = np.ascontiguousarray(x[b].astype(np.float16).T)   # [C, N]
        per_core.append({"xT": xTb, "wqk": wqk, "wv": wv, "wpt": wpt,
                         "bias": bias})
    return per_core


def kernel(x, Wqkv, Wproj, bproj, Aq, Bq, Av, Bv, _trace=False):
    x = np.asarray(x)
    in_maps = _host_prep(np.asarray(x), np.asarray(Wqkv), np.asarray(Wproj),
                         np.asarray(bproj), np.asarray(Aq), np.asarray(Bq),
                         np.asarray(Av), np.asarray(Bv))
    nc = _get_nc()
    res = run_bass_kernel_spmd(nc, in_maps, core_ids=list(range(N_CORES)),
                               trace=_trace)
    out = np.stack([np.asarray(res.results[b]["y"]) for b in range(B)], axis=0)
    if _trace:
        kernel._last_result = res
    return out.astype(np.float32)
